# revision 9
# baseline (speedup 1.0000x reference)
"""MMIF fusion network as a Trainium2 Bass kernel, 8-way data parallel.

The reference applies a 2D fusion LUT to (A,B), chains four spatial-detail 3D
LUTs, applies 1D tone curves, and converts YCbCr->RGB.  The LUT tables arrive
as runtime inputs; this kernel inspects them on the host (they are tiny) and
proves they are affine in their grid indices:

  * multilinear interpolation of a table that is affine in its indices is an
    affine function of the (clipped) coordinates, so each LUT stage collapses
    to scale/bias;
  * the 3D tables must additionally be pass-through in the center coordinate
    (t[i,j,k] == i/(D-1)), which makes every sdlut stage the identity on
    already-clipped data and removes the neighbor-roll dependency entirely.

Everything then folds into 12 scalars and the device work is a pure streaming
elementwise pipeline: DVE + ACT ops over [128, 1024] fp32 tiles, 8 H-shards.
If the tables ever fail the structure check, a bit-faithful numpy fallback
computes the reference directly (never taken for the real problem inputs).
"""

import numpy as np
from contextlib import ExitStack

import concourse.bass as bass
import concourse.tile as tile
from concourse import bacc, mybir
from concourse import bass_utils

F32 = mybir.dt.float32
ALU = mybir.AluOpType
ACTF = mybir.ActivationFunctionType

NCORES = 8
B, H, W = 2, 2048, 2048
HS = H // NCORES              # 256 rows per core
N = B * HS * W                # 1,048,576 floats per tensor per core
P = 128

# tuning knobs (read by _build_nc; estimate.py sweeps these)
TUNE = dict(C=1024, bufs_in=3, bufs_out=3, bufs_tmp=2, pool_adds=0,
            dma_only=False, reps=1, taper=0)


def _affine_fit_1d(t, tol=1e-6):
    """t[i] ~= a + s*i ?  ->  (a, s) in float64, or None."""
    t = np.asarray(t, np.float64)
    d = t.shape[0]
    i = np.arange(d)
    a = t[0]
    s = (t[-1] - t[0]) / (d - 1)
    if np.abs(t - (a + s * i)).max() > tol:
        return None
    return a, s


def _affine_fit_2d(t, tol=1e-6):
    t = np.asarray(t, np.float64)
    d0, d1 = t.shape
    i = np.arange(d0)[:, None]
    j = np.arange(d1)[None, :]
    a = t[0, 0]
    di = (t[-1, 0] - t[0, 0]) / (d0 - 1)
    dj = (t[0, -1] - t[0, 0]) / (d1 - 1)
    if np.abs(t - (a + di * i + dj * j)).max() > tol:
        return None
    return a, di, dj


def _is_center_passthrough_3d(t, tol=1e-7):
    """t[i,j,k] == i/(D-1) for all j,k (the sdlut stage is then identity on
    clipped input)."""
    t = np.asarray(t, np.float64)
    d = t.shape[0]
    g = (np.arange(d) / (d - 1))[:, None, None]
    return np.abs(t - g).max() <= tol


def _fold_constants(lut2d, lut3ds, lut1d_pgf, lut1d_cb, lut1d_cr):
    f2 = _affine_fit_2d(lut2d)
    if f2 is None:
        return None
    for t3 in lut3ds:
        if not _is_center_passthrough_3d(t3):
            return None
    f1p = _affine_fit_1d(lut1d_pgf)
    f1b = _affine_fit_1d(lut1d_cb)
    f1r = _affine_fit_1d(lut1d_cr)
    if f1p is None or f1b is None or f1r is None:
        return None

    t00, di, dj = f2
    d2 = np.asarray(lut2d).shape[0] - 1          # 32
    d1p = np.asarray(lut1d_pgf).shape[0] - 1     # 255
    a1, s1 = f1p
    a2, s2 = _affine_fit_1d(lut1d_cb)
    a3, s3 = f1r
    B1 = d1p * s1        # pg1 = a1 + B1*pg0
    B2 = d1p * s2        # fcb = a2 + B2*cb
    B3 = d1p * s3        # fcr = a3 + B3*cr
    if abs(B1) < 1e-12:
        return None      # degenerate; use fallback

    half = 128.0 / 256.0
    k = dict(
        # pre-clip fused-plane, scaled by B1:  pp = B1*(t00 + d2*di*A + d2*dj*B)
        kA=B1 * d2 * di,
        kB=B1 * d2 * dj,
        kb0=B1 * t00,
        B1=B1,
        # out0 = q + m_cr0*cr + g0
        m_cr0=1.402 * B3,
        g0=a1 + 1.402 * (a3 - half),
        # out1 = q + m_cb1*cb + m_cr1*cr + g1
        m_cb1=-0.344136 * B2,
        m_cr1=-0.714136 * B3,
        g1=a1 - 0.344136 * (a2 - half) - 0.714136 * (a3 - half),
        # out2 = q + m_cb2*cb + g2
        m_cb2=1.772 * B2,
        g2=a1 + 1.772 * (a2 - half),
    )
    return {kk: float(np.float32(v)) for kk, v in k.items()}


def _build_nc(k, tune=None):
    t_ = dict(TUNE)
    if tune:
        t_.update(tune)
    C = t_["C"]
    pool_adds = t_["pool_adds"]  # how many of the 3 output adds go to GPSIMD
    F = N // P                   # free-dim length per partition (8192)

    # column chunks over the [128, F] view; taper=s shrinks the pipeline
    # head/tail into chunks of size s for faster fill and a shorter exposed
    # drain
    taper = t_["taper"]
    chunks = []
    if taper:
        assert C % taper == 0 and taper * 2 <= C
        chunks += [taper] * (C // taper)
        body = (F - 2 * C) // C
        chunks += [C] * body
        chunks += [taper] * (C // taper)
        assert sum(chunks) == F
    else:
        chunks = [C] * (F // C)

    # when the 2D-LUT slopes match, A+B can be summed before scaling: one
    # tensor_tensor instead of scale+scale+add, and a shorter critical path
    merge_ab = abs(k["kA"] - k["kB"]) <= 1e-9 * max(abs(k["kA"]), 1e-30)

    nc = bacc.Bacc("TRN2", target_bir_lowering=False, debug=False,
                   num_devices=NCORES)
    dram = {}
    for name in ("a", "b", "cb", "cr"):
        dram[name] = nc.dram_tensor(name, [N], F32, kind="ExternalInput").ap()
    for name in ("o0", "o1", "o2"):
        dram[name] = nc.dram_tensor(name, [N], F32, kind="ExternalOutput").ap()
    rs = lambda t: t.rearrange("(p f) -> p f", p=P)
    A, Bt, CB, CR, O0, O1, O2 = (rs(dram[n]) for n in
                                 ("a", "b", "cb", "cr", "o0", "o1", "o2"))

    B1 = k["B1"]
    qlo, qhi = (0.0, B1) if B1 > 0 else (B1, 0.0)

    with tile.TileContext(nc) as tc, ExitStack() as ctx:
        pin = ctx.enter_context(tc.tile_pool(name="pin", bufs=t_["bufs_in"]))
        pout = ctx.enter_context(tc.tile_pool(name="pout", bufs=t_["bufs_out"]))
        ptmp = ctx.enter_context(tc.tile_pool(name="ptmp", bufs=t_["bufs_tmp"]))

        for _ in range(t_["reps"]):
            off = 0
            for c in chunks:
                sl = slice(off, off + c)
                off += c

                ta = pin.tile([P, c], F32, tag="a")
                nc.sync.dma_start(ta[:], A[:, sl])
                tb = pin.tile([P, c], F32, tag="b")
                nc.sync.dma_start(tb[:], Bt[:, sl])
                tcb = pin.tile([P, c], F32, tag="cb")
                nc.sync.dma_start(tcb[:], CB[:, sl])
                tcr = pin.tile([P, c], F32, tag="cr")
                nc.sync.dma_start(tcr[:], CR[:, sl])

                if t_["dma_only"]:
                    nc.sync.dma_start(O0[:, sl], ta[:])
                    nc.sync.dma_start(O2[:, sl], tb[:])
                    nc.sync.dma_start(O1[:, sl], tcb[:])
                    continue

                def out_add(dst, src0, src1, idx):
                    if idx < pool_adds:
                        nc.gpsimd.tensor_tensor(dst, src0, src1, ALU.add)
                    else:
                        nc.vector.tensor_tensor(dst, src0, src1, ALU.add)

                # q = B1 * clip01(t00 + d2*(di*A + dj*B))
                if merge_ab:
                    s = ptmp.tile([P, c], F32, tag="sb")
                    nc.vector.tensor_tensor(s[:], ta[:], tb[:], ALU.add)
                    pp = ptmp.tile([P, c], F32, tag="pp")
                    nc.scalar.activation(pp[:], s[:], ACTF.Copy,
                                         bias=k["kb0"], scale=k["kA"])
                else:
                    sa = ptmp.tile([P, c], F32, tag="sa")
                    nc.vector.tensor_scalar(sa[:], ta[:], k["kA"], None,
                                            op0=ALU.mult)
                    sb = ptmp.tile([P, c], F32, tag="sb")
                    nc.scalar.activation(sb[:], tb[:], ACTF.Copy,
                                         bias=k["kb0"], scale=k["kB"])
                    pp = ptmp.tile([P, c], F32, tag="pp")
                    nc.vector.tensor_tensor(pp[:], sa[:], sb[:], ALU.add)
                q = ptmp.tile([P, c], F32, tag="q")
                nc.vector.tensor_scalar(q[:], pp[:], qlo, qhi,
                                        op0=ALU.max, op1=ALU.min)

                # out0 = q + (m_cr0*cr + g0)
                t0 = pout.tile([P, c], F32, tag="o0")
                nc.scalar.activation(t0[:], tcr[:], ACTF.Copy,
                                     bias=k["g0"], scale=k["m_cr0"])
                out_add(t0[:], q[:], t0[:], 0)
                nc.sync.dma_start(O0[:, sl], t0[:])

                # out2 = q + (m_cb2*cb + g2)
                t2 = pout.tile([P, c], F32, tag="o2")
                nc.scalar.activation(t2[:], tcb[:], ACTF.Copy,
                                     bias=k["g2"], scale=k["m_cb2"])
                out_add(t2[:], q[:], t2[:], 1)
                nc.sync.dma_start(O2[:, sl], t2[:])

                # out1 = q + (m_cb1*cb + g1) + m_cr1*cr
                t1 = pout.tile([P, c], F32, tag="o1")
                nc.scalar.activation(t1[:], tcb[:], ACTF.Copy,
                                     bias=k["g1"], scale=k["m_cb1"])
                c1u = ptmp.tile([P, c], F32, tag="c1u")
                nc.vector.tensor_scalar(c1u[:], tcr[:], k["m_cr1"], None,
                                        op0=ALU.mult)
                nc.vector.tensor_tensor(t1[:], t1[:], c1u[:], ALU.add)
                out_add(t1[:], q[:], t1[:], 2)
                nc.sync.dma_start(O1[:, sl], t1[:])

    nc.compile()
    return nc


_NC_CACHE = {}


def _get_nc(k):
    key = tuple(sorted(k.items()))
    if key not in _NC_CACHE:
        _NC_CACHE[key] = _build_nc(k)
    return _NC_CACHE[key]


# ---------------------------------------------------------------- fallback


def _np_idx(x, d):
    xs = np.clip(x, 0.0, 1.0).astype(np.float32) * np.float32(d - 1)
    i0 = np.clip(np.floor(xs), 0, d - 2).astype(np.int32)
    return i0, xs - i0.astype(np.float32)


def _np_lut1d(t, x):
    i, f = _np_idx(x, t.shape[0])
    return t[i] * (1 - f) + t[i + 1] * f


def _np_lut2d(t, u, v):
    i, fi = _np_idx(u, t.shape[0])
    j, fj = _np_idx(v, t.shape[1])
    return (t[i, j] * (1 - fi) * (1 - fj) + t[i + 1, j] * fi * (1 - fj)
            + t[i, j + 1] * (1 - fi) * fj + t[i + 1, j + 1] * fi * fj)


def _np_lut3d(t, u, v, w):
    i, fi = _np_idx(u, t.shape[0])
    j, fj = _np_idx(v, t.shape[1])
    k, fk = _np_idx(w, t.shape[2])
    return (t[i, j, k] * (1 - fi) * (1 - fj) * (1 - fk)
            + t[i + 1, j, k] * fi * (1 - fj) * (1 - fk)
            + t[i, j + 1, k] * (1 - fi) * fj * (1 - fk)
            + t[i, j, k + 1] * (1 - fi) * (1 - fj) * fk
            + t[i + 1, j + 1, k] * fi * fj * (1 - fk)
            + t[i + 1, j, k + 1] * fi * (1 - fj) * fk
            + t[i, j + 1, k + 1] * (1 - fi) * fj * fk
            + t[i + 1, j + 1, k + 1] * fi * fj * fk)


def _np_sdlut(t, x, dy, dx):
    n1 = np.roll(x, (dy, dx), axis=(-2, -1))
    n2 = np.roll(x, (2 * dy, 2 * dx), axis=(-2, -1))
    return _np_lut3d(t, x, n1, n2)


def _np_reference(A_image, B_image, cb, cr, lut2d, lut3d_0, lut3d_90,
                  lut3d_180, lut3d_270, lut1d_pgf, lut1d_cb, lut1d_cr):
    pg0 = _np_lut2d(lut2d, A_image[:, 0], B_image[:, 0])[:, None]
    pg0 = np.clip(pg0, 0.0, 1.0).transpose(1, 0, 2, 3)
    sd = np.clip(_np_sdlut(lut3d_0, pg0, 0, -1), 0.0, 1.0)
    sd = np.clip(_np_sdlut(lut3d_90, sd, 1, 0), 0.0, 1.0)
    sd = np.clip(_np_sdlut(lut3d_180, sd, 0, 1), 0.0, 1.0)
    sd = np.clip(_np_sdlut(lut3d_270, sd, -1, 0), 0.0, 1.0).transpose(1, 0, 2, 3)
    pg1 = _np_lut1d(lut1d_pgf, sd)
    fcb = _np_lut1d(lut1d_cb, cb)
    fcr = _np_lut1d(lut1d_cr, cr)
    y, cbc, crc = pg1[:, 0], fcb[:, 0], fcr[:, 0]
    half = np.float32(0.5)
    out = np.stack((y + (crc - half) * np.float32(1.402),
                    y - (cbc - half) * np.float32(0.344136)
                      - (crc - half) * np.float32(0.714136),
                    y + (cbc - half) * np.float32(1.772)), axis=1)
    return out.astype(np.float32)


# ---------------------------------------------------------------- entry


def kernel(A_image, B_image, cb, cr, lut2d, lut3d_0, lut3d_90, lut3d_180,
           lut3d_270, lut1d_pgf, lut1d_cb, lut1d_cr):
    args = dict(A_image=np.asarray(A_image), B_image=np.asarray(B_image),
                cb=np.asarray(cb), cr=np.asarray(cr),
                lut2d=np.asarray(lut2d), lut3d_0=np.asarray(lut3d_0),
                lut3d_90=np.asarray(lut3d_90), lut3d_180=np.asarray(lut3d_180),
                lut3d_270=np.asarray(lut3d_270),
                lut1d_pgf=np.asarray(lut1d_pgf),
                lut1d_cb=np.asarray(lut1d_cb), lut1d_cr=np.asarray(lut1d_cr))

    k = _fold_constants(args["lut2d"],
                        [args["lut3d_0"], args["lut3d_90"],
                         args["lut3d_180"], args["lut3d_270"]],
                        args["lut1d_pgf"], args["lut1d_cb"], args["lut1d_cr"])
    if k is None or args["A_image"].shape != (B, 1, H, W):
        return _np_reference(**args)

    nc = _get_nc(k)

    in_maps = []
    for c in range(NCORES):
        sl = slice(c * HS, (c + 1) * HS)
        in_maps.append({
            "a": np.ascontiguousarray(args["A_image"][:, 0, sl, :],
                                      np.float32).reshape(N),
            "b": np.ascontiguousarray(args["B_image"][:, 0, sl, :],
                                      np.float32).reshape(N),
            "cb": np.ascontiguousarray(args["cb"][:, 0, sl, :],
                                       np.float32).reshape(N),
            "cr": np.ascontiguousarray(args["cr"][:, 0, sl, :],
                                       np.float32).reshape(N),
        })

    res = bass_utils.run_bass_kernel_spmd(nc, in_maps,
                                          core_ids=list(range(NCORES)))

    out = np.empty((B, 3, H, W), np.float32)
    for c in range(NCORES):
        sl = slice(c * HS, (c + 1) * HS)
        r = res.results[c]
        out[:, 0, sl, :] = r["o0"].reshape(B, HS, W)
        out[:, 1, sl, :] = r["o1"].reshape(B, HS, W)
        out[:, 2, sl, :] = r["o2"].reshape(B, HS, W)
    return out


# revision 16
# speedup vs baseline: 3.3085x; 3.3085x over previous
"""MMIF fusion network as a Trainium2 Bass kernel, 8-way data parallel.

The reference applies a 2D fusion LUT to (A,B), chains four spatial-detail 3D
LUTs, applies 1D tone curves, and converts YCbCr->RGB.  The LUT tables arrive
as runtime inputs; this kernel inspects them on the host (they are tiny) and
proves they are affine in their grid indices:

  * multilinear interpolation of a table that is affine in its indices is an
    affine function of the (clipped) coordinates, so each LUT stage collapses
    to scale/bias;
  * the 3D tables must additionally be pass-through in the center coordinate
    (t[i,j,k] == i/(D-1)), which makes every sdlut stage the identity on
    already-clipped data and removes the neighbor-roll dependency entirely.

Everything then folds into 12 scalars and the device work is a pure streaming
elementwise pipeline: DVE + ACT ops over [128, 1024] fp32 tiles, 8 H-shards.
If the tables ever fail the structure check, a bit-faithful numpy fallback
computes the reference directly (never taken for the real problem inputs).
"""

import numpy as np
from contextlib import ExitStack

import concourse.bass as bass
import concourse.tile as tile
from concourse import bacc, mybir
from concourse import bass_utils

F32 = mybir.dt.float32
ALU = mybir.AluOpType
ACTF = mybir.ActivationFunctionType

NCORES = 8
B, H, W = 2, 2048, 2048
HS = H // NCORES              # 256 rows per core
N = B * HS * W                # 1,048,576 floats per tensor per core
P = 128

# tuning knobs (read by _build_nc; estimate.py sweeps these)
TUNE = dict(C=1024, bufs_in=3, bufs_out=3, bufs_tmp=2, pool_adds=0,
            dma_only=False, reps=1, taper=0, tail_split=1, o1_first=False,
            hw_loop_reps=1)


def _affine_fit_1d(t, tol=1e-6):
    """t[i] ~= a + s*i ?  ->  (a, s) in float64, or None."""
    t = np.asarray(t, np.float64)
    d = t.shape[0]
    i = np.arange(d)
    a = t[0]
    s = (t[-1] - t[0]) / (d - 1)
    if np.abs(t - (a + s * i)).max() > tol:
        return None
    return a, s


def _affine_fit_2d(t, tol=1e-6):
    t = np.asarray(t, np.float64)
    d0, d1 = t.shape
    i = np.arange(d0)[:, None]
    j = np.arange(d1)[None, :]
    a = t[0, 0]
    di = (t[-1, 0] - t[0, 0]) / (d0 - 1)
    dj = (t[0, -1] - t[0, 0]) / (d1 - 1)
    if np.abs(t - (a + di * i + dj * j)).max() > tol:
        return None
    return a, di, dj


def _is_center_passthrough_3d(t, tol=1e-7):
    """t[i,j,k] == i/(D-1) for all j,k (the sdlut stage is then identity on
    clipped input)."""
    t = np.asarray(t, np.float64)
    d = t.shape[0]
    g = (np.arange(d) / (d - 1))[:, None, None]
    return np.abs(t - g).max() <= tol


def _fold_constants(lut2d, lut3ds, lut1d_pgf, lut1d_cb, lut1d_cr):
    f2 = _affine_fit_2d(lut2d)
    if f2 is None:
        return None
    for t3 in lut3ds:
        if not _is_center_passthrough_3d(t3):
            return None
    f1p = _affine_fit_1d(lut1d_pgf)
    f1b = _affine_fit_1d(lut1d_cb)
    f1r = _affine_fit_1d(lut1d_cr)
    if f1p is None or f1b is None or f1r is None:
        return None

    t00, di, dj = f2
    d2 = np.asarray(lut2d).shape[0] - 1          # 32
    d1p = np.asarray(lut1d_pgf).shape[0] - 1     # 255
    a1, s1 = f1p
    a2, s2 = _affine_fit_1d(lut1d_cb)
    a3, s3 = f1r
    B1 = d1p * s1        # pg1 = a1 + B1*pg0
    B2 = d1p * s2        # fcb = a2 + B2*cb
    B3 = d1p * s3        # fcr = a3 + B3*cr
    if abs(B1) < 1e-12:
        return None      # degenerate; use fallback

    half = 128.0 / 256.0
    k = dict(
        # pre-clip fused-plane, scaled by B1:  pp = B1*(t00 + d2*di*A + d2*dj*B)
        kA=B1 * d2 * di,
        kB=B1 * d2 * dj,
        kb0=B1 * t00,
        B1=B1,
        # out0 = q + m_cr0*cr + g0
        m_cr0=1.402 * B3,
        g0=a1 + 1.402 * (a3 - half),
        # out1 = q + m_cb1*cb + m_cr1*cr + g1
        m_cb1=-0.344136 * B2,
        m_cr1=-0.714136 * B3,
        g1=a1 - 0.344136 * (a2 - half) - 0.714136 * (a3 - half),
        # out2 = q + m_cb2*cb + g2
        m_cb2=1.772 * B2,
        g2=a1 + 1.772 * (a2 - half),
    )
    return {kk: float(np.float32(v)) for kk, v in k.items()}


def _build_nc(k, tune=None):
    t_ = dict(TUNE)
    if tune:
        t_.update(tune)
    C = t_["C"]
    pool_adds = t_["pool_adds"]  # how many of the 3 output adds go to GPSIMD
    F = N // P                   # free-dim length per partition (8192)

    # column chunks over the [128, F] view; taper=s shrinks the pipeline
    # head/tail into chunks of size s for faster fill and a shorter exposed
    # drain
    taper = t_["taper"]
    chunks = []
    if taper:
        assert C % taper == 0 and taper * 2 <= C
        chunks += [taper] * (C // taper)
        body = (F - 2 * C) // C
        chunks += [C] * body
        chunks += [taper] * (C // taper)
        assert sum(chunks) == F
    else:
        chunks = [C] * (F // C)
    ts_ = t_["tail_split"]
    if ts_ > 1:
        chunks = chunks[:-1] + [chunks[-1] // ts_] * ts_
        assert sum(chunks) == F

    # when the 2D-LUT slopes match, A+B can be summed before scaling: one
    # tensor_tensor instead of scale+scale+add, and a shorter critical path
    merge_ab = abs(k["kA"] - k["kB"]) <= 1e-9 * max(abs(k["kA"]), 1e-30)

    nc = bacc.Bacc("TRN2", target_bir_lowering=False, debug=False,
                   num_devices=NCORES)
    dram = {}
    for name in ("a", "b", "cb", "cr"):
        dram[name] = nc.dram_tensor(name, [N], F32, kind="ExternalInput").ap()
    for name in ("o0", "o1", "o2"):
        dram[name] = nc.dram_tensor(name, [N], F32, kind="ExternalOutput").ap()
    rs = lambda t: t.rearrange("(p f) -> p f", p=P)
    A, Bt, CB, CR, O0, O1, O2 = (rs(dram[n]) for n in
                                 ("a", "b", "cb", "cr", "o0", "o1", "o2"))

    B1 = k["B1"]
    qlo, qhi = (0.0, B1) if B1 > 0 else (B1, 0.0)

    with tile.TileContext(nc) as tc, ExitStack() as ctx:
        pin = ctx.enter_context(tc.tile_pool(name="pin", bufs=t_["bufs_in"]))
        pout = ctx.enter_context(tc.tile_pool(name="pout", bufs=t_["bufs_out"]))
        ptmp = ctx.enter_context(tc.tile_pool(name="ptmp", bufs=t_["bufs_tmp"]))

        loop_ctx = (tc.For_i(0, t_["hw_loop_reps"], 1)
                    if t_["hw_loop_reps"] > 1 else None)
        if loop_ctx is not None:
            ctx.enter_context(loop_ctx)
        for _ in range(t_["reps"]):
            off = 0
            for c in chunks:
                sl = slice(off, off + c)
                off += c

                ta = pin.tile([P, c], F32, tag="a")
                nc.sync.dma_start(ta[:], A[:, sl])
                tb = pin.tile([P, c], F32, tag="b")
                nc.sync.dma_start(tb[:], Bt[:, sl])
                tcb = pin.tile([P, c], F32, tag="cb")
                nc.sync.dma_start(tcb[:], CB[:, sl])
                tcr = pin.tile([P, c], F32, tag="cr")
                nc.sync.dma_start(tcr[:], CR[:, sl])

                if t_["dma_only"]:
                    nc.sync.dma_start(O0[:, sl], ta[:])
                    nc.sync.dma_start(O2[:, sl], tb[:])
                    nc.sync.dma_start(O1[:, sl], tcb[:])
                    continue

                def out_add(dst, src0, src1, idx):
                    if idx < pool_adds:
                        nc.gpsimd.tensor_tensor(dst, src0, src1, ALU.add)
                    else:
                        nc.vector.tensor_tensor(dst, src0, src1, ALU.add)

                # q = B1 * clip01(t00 + d2*(di*A + dj*B))
                if merge_ab:
                    s = ptmp.tile([P, c], F32, tag="sb")
                    nc.vector.tensor_tensor(s[:], ta[:], tb[:], ALU.add)
                    pp = ptmp.tile([P, c], F32, tag="pp")
                    nc.scalar.activation(pp[:], s[:], ACTF.Copy,
                                         bias=k["kb0"], scale=k["kA"])
                else:
                    sa = ptmp.tile([P, c], F32, tag="sa")
                    nc.vector.tensor_scalar(sa[:], ta[:], k["kA"], None,
                                            op0=ALU.mult)
                    sb = ptmp.tile([P, c], F32, tag="sb")
                    nc.scalar.activation(sb[:], tb[:], ACTF.Copy,
                                         bias=k["kb0"], scale=k["kB"])
                    pp = ptmp.tile([P, c], F32, tag="pp")
                    nc.vector.tensor_tensor(pp[:], sa[:], sb[:], ALU.add)
                q = ptmp.tile([P, c], F32, tag="q")
                nc.vector.tensor_scalar(q[:], pp[:], qlo, qhi,
                                        op0=ALU.max, op1=ALU.min)

                def emit_o0():
                    # out0 = q + (m_cr0*cr + g0)
                    t0 = pout.tile([P, c], F32, tag="o0")
                    nc.scalar.activation(t0[:], tcr[:], ACTF.Copy,
                                         bias=k["g0"], scale=k["m_cr0"])
                    out_add(t0[:], q[:], t0[:], 0)
                    nc.sync.dma_start(O0[:, sl], t0[:])

                def emit_o2():
                    # out2 = q + (m_cb2*cb + g2)
                    t2 = pout.tile([P, c], F32, tag="o2")
                    nc.scalar.activation(t2[:], tcb[:], ACTF.Copy,
                                         bias=k["g2"], scale=k["m_cb2"])
                    out_add(t2[:], q[:], t2[:], 1)
                    nc.sync.dma_start(O2[:, sl], t2[:])

                def emit_o1():
                    # out1 = q + (m_cb1*cb + g1) + m_cr1*cr
                    t1 = pout.tile([P, c], F32, tag="o1")
                    nc.scalar.activation(t1[:], tcb[:], ACTF.Copy,
                                         bias=k["g1"], scale=k["m_cb1"])
                    c1u = ptmp.tile([P, c], F32, tag="c1u")
                    nc.vector.tensor_scalar(c1u[:], tcr[:], k["m_cr1"], None,
                                            op0=ALU.mult)
                    nc.vector.tensor_tensor(t1[:], t1[:], c1u[:], ALU.add)
                    out_add(t1[:], q[:], t1[:], 2)
                    nc.sync.dma_start(O1[:, sl], t1[:])

                order = ((emit_o1, emit_o0, emit_o2) if t_["o1_first"]
                         else (emit_o0, emit_o2, emit_o1))
                for emit in order:
                    emit()

    nc.compile()
    return nc


_NC_CACHE = {}


def _get_nc(k):
    key = tuple(sorted(k.items()))
    if key not in _NC_CACHE:
        _NC_CACHE[key] = _build_nc(k)
    return _NC_CACHE[key]


# ---------------------------------------------------------------- fallback


def _np_idx(x, d):
    xs = np.clip(x, 0.0, 1.0).astype(np.float32) * np.float32(d - 1)
    i0 = np.clip(np.floor(xs), 0, d - 2).astype(np.int32)
    return i0, xs - i0.astype(np.float32)


def _np_lut1d(t, x):
    i, f = _np_idx(x, t.shape[0])
    return t[i] * (1 - f) + t[i + 1] * f


def _np_lut2d(t, u, v):
    i, fi = _np_idx(u, t.shape[0])
    j, fj = _np_idx(v, t.shape[1])
    return (t[i, j] * (1 - fi) * (1 - fj) + t[i + 1, j] * fi * (1 - fj)
            + t[i, j + 1] * (1 - fi) * fj + t[i + 1, j + 1] * fi * fj)


def _np_lut3d(t, u, v, w):
    i, fi = _np_idx(u, t.shape[0])
    j, fj = _np_idx(v, t.shape[1])
    k, fk = _np_idx(w, t.shape[2])
    return (t[i, j, k] * (1 - fi) * (1 - fj) * (1 - fk)
            + t[i + 1, j, k] * fi * (1 - fj) * (1 - fk)
            + t[i, j + 1, k] * (1 - fi) * fj * (1 - fk)
            + t[i, j, k + 1] * (1 - fi) * (1 - fj) * fk
            + t[i + 1, j + 1, k] * fi * fj * (1 - fk)
            + t[i + 1, j, k + 1] * fi * (1 - fj) * fk
            + t[i, j + 1, k + 1] * (1 - fi) * fj * fk
            + t[i + 1, j + 1, k + 1] * fi * fj * fk)


def _np_sdlut(t, x, dy, dx):
    n1 = np.roll(x, (dy, dx), axis=(-2, -1))
    n2 = np.roll(x, (2 * dy, 2 * dx), axis=(-2, -1))
    return _np_lut3d(t, x, n1, n2)


def _np_reference(A_image, B_image, cb, cr, lut2d, lut3d_0, lut3d_90,
                  lut3d_180, lut3d_270, lut1d_pgf, lut1d_cb, lut1d_cr):
    pg0 = _np_lut2d(lut2d, A_image[:, 0], B_image[:, 0])[:, None]
    pg0 = np.clip(pg0, 0.0, 1.0).transpose(1, 0, 2, 3)
    sd = np.clip(_np_sdlut(lut3d_0, pg0, 0, -1), 0.0, 1.0)
    sd = np.clip(_np_sdlut(lut3d_90, sd, 1, 0), 0.0, 1.0)
    sd = np.clip(_np_sdlut(lut3d_180, sd, 0, 1), 0.0, 1.0)
    sd = np.clip(_np_sdlut(lut3d_270, sd, -1, 0), 0.0, 1.0).transpose(1, 0, 2, 3)
    pg1 = _np_lut1d(lut1d_pgf, sd)
    fcb = _np_lut1d(lut1d_cb, cb)
    fcr = _np_lut1d(lut1d_cr, cr)
    y, cbc, crc = pg1[:, 0], fcb[:, 0], fcr[:, 0]
    half = np.float32(0.5)
    out = np.stack((y + (crc - half) * np.float32(1.402),
                    y - (cbc - half) * np.float32(0.344136)
                      - (crc - half) * np.float32(0.714136),
                    y + (cbc - half) * np.float32(1.772)), axis=1)
    return out.astype(np.float32)


# ---------------------------------------------------------------- entry


def kernel(A_image, B_image, cb, cr, lut2d, lut3d_0, lut3d_90, lut3d_180,
           lut3d_270, lut1d_pgf, lut1d_cb, lut1d_cr):
    args = dict(A_image=np.asarray(A_image), B_image=np.asarray(B_image),
                cb=np.asarray(cb), cr=np.asarray(cr),
                lut2d=np.asarray(lut2d), lut3d_0=np.asarray(lut3d_0),
                lut3d_90=np.asarray(lut3d_90), lut3d_180=np.asarray(lut3d_180),
                lut3d_270=np.asarray(lut3d_270),
                lut1d_pgf=np.asarray(lut1d_pgf),
                lut1d_cb=np.asarray(lut1d_cb), lut1d_cr=np.asarray(lut1d_cr))

    k = _fold_constants(args["lut2d"],
                        [args["lut3d_0"], args["lut3d_90"],
                         args["lut3d_180"], args["lut3d_270"]],
                        args["lut1d_pgf"], args["lut1d_cb"], args["lut1d_cr"])
    if k is None or args["A_image"].shape != (B, 1, H, W):
        return _np_reference(**args)

    # the device pipeline folds away the reference's clip01 on raw inputs,
    # which is only exact for in-range data (uniform [0,1) per the problem
    # spec); verify and fall back otherwise (also catches NaN)
    for name in ("A_image", "B_image", "cb", "cr"):
        x = args[name]
        if not (np.min(x) >= 0.0 and np.max(x) <= 1.0):
            return _np_reference(**args)

    try:
        nc = _get_nc(k)

        in_maps = []
        for c in range(NCORES):
            sl = slice(c * HS, (c + 1) * HS)
            in_maps.append({
                "a": np.ascontiguousarray(args["A_image"][:, 0, sl, :],
                                          np.float32).reshape(N),
                "b": np.ascontiguousarray(args["B_image"][:, 0, sl, :],
                                          np.float32).reshape(N),
                "cb": np.ascontiguousarray(args["cb"][:, 0, sl, :],
                                           np.float32).reshape(N),
                "cr": np.ascontiguousarray(args["cr"][:, 0, sl, :],
                                           np.float32).reshape(N),
            })

        res = bass_utils.run_bass_kernel_spmd(nc, in_maps,
                                              core_ids=list(range(NCORES)))
    except Exception as e:
        import traceback
        traceback.print_exc()
        print(f"kernel: device path failed ({type(e).__name__}); "
              f"using host fallback")
        return _np_reference(**args)

    out = np.empty((B, 3, H, W), np.float32)
    for c in range(NCORES):
        sl = slice(c * HS, (c + 1) * HS)
        r = res.results[c]
        out[:, 0, sl, :] = r["o0"].reshape(B, HS, W)
        out[:, 1, sl, :] = r["o1"].reshape(B, HS, W)
        out[:, 2, sl, :] = r["o2"].reshape(B, HS, W)
    return out
